# revision 29
# baseline (speedup 1.0000x reference)
"""Trainium2 Bass kernel for nn_HadamardTransform: Y = X @ H4096_normalized.

Algorithm: H4096 (Sylvester, normalized) factors exactly as the Kronecker
product H16n (x) H256n.  Each row x of X, reshaped row-major to R[16, 256],
transforms as  Y_mat = G16 @ R @ H256u  with G16 = 2^-6 * H16u (all of the
2^-6 normalization folded into the 16-side so H256u stays exactly +-1).

On-chip scheme per 8-row slice (partition p = 16*b + ip, b in [8] rows,
ip in [16] column superblocks of 256; free = (a, jc in [256])):
  MM-A (x2, one per 128-half jh of jc):
      psumA[jh-half] = T_jh.T @ W1      (W1 = I8 (x) G16, block-diagonal)
      -> psumA[jc, (b,i')] : the 16-transform, emerging jc-on-partitions
      (the two jc halves land in different psum tiles' partition meaning,
       packed as columns [0:128]/[128:256] of one psum bank)
  MM-B (x4, accumulating pairs): psumB[:, ch] = sum_h sah.T @ H256[h, ch]
      -> psumB[(b,i'), jc'] : the 256-transform, natural output layout
No transposes are needed anywhere; the fixed matrices are the moving
operands, the per-slice data is the stationary operand.

Why 16x256 rather than 32x128: the DMA descriptor economy.  Both layouts
move the same bytes, but contiguous runs here are 512 B (256 fp16) vs
256 B, halving descriptor count; measured on HW the 8-core DMA floor
drops from ~67 us to ~60.5 us (payload-bound at ~278 GB/s/core), at the
cost of a third PE pass (the 256-contraction needs two accumulating
matmuls).  PE and DMA both land near 60 us and overlap.

Everything on device is fp16: X is cast host-side (rel err ~5e-4 vs the
2e-2 gate), halving DMA traffic (the roofline here); fp16 moving
operands stream at 1 cycle/column through the PE vs 4 for fp32.  PSUM
accumulation stays fp32; the PSUM->SBUF evacuations cast to fp16 (DVE
for stage A, ACT for stage B).  Loads ride the SP HWDGE ring, stores the
ACT HWDGE ring.

Sharding: X's 8192 rows split into 8 contiguous shards of 1024 rows, one
per NeuronCore (pure data parallelism, no collectives).
"""

import sys

import numpy as np

try:
    import concourse.bass as bass
except ImportError:  # repo not on sys.path in a fresh grading dir
    sys.path.insert(0, "/opt/trn_rl_repo")
    import concourse.bass as bass

import concourse.mybir as mybir
import concourse.tile as tile
from concourse import bacc
from concourse.bass_utils import run_bass_kernel_spmd

N_CORES = 8
ROWS = 8192
N = 4096
ROWS_PER_CORE = ROWS // N_CORES  # 1024
ROWS_PER_GROUP = 64               # rows moved per DMA (512 KiB fp16)
A_PER_GROUP = ROWS_PER_GROUP // 8  # 8-row slices per group
GROUPS = ROWS_PER_CORE // ROWS_PER_GROUP  # 16
F16 = mybir.dt.float16
F32 = mybir.dt.float32


def _hadamard_u(n: int) -> np.ndarray:
    """Unnormalized Sylvester Hadamard matrix (+-1 entries)."""
    H = np.array([[1.0]], dtype=np.float64)
    while H.shape[0] < n:
        H = np.block([[H, H], [H, -H]])
    return H


def _constants() -> tuple[np.ndarray, np.ndarray]:
    G16 = (2.0 ** -6) * _hadamard_u(16)         # fold full 2^-6 norm here
    W1 = np.kron(np.eye(8), G16).astype(np.float16)  # [128,128] block-diag
    HJ2 = _hadamard_u(256).astype(np.float16)        # [256,256] exact +-1
    return W1, HJ2


def _build_bass(loop_reps: int | None = None, mode: str = "full"):
    """loop_reps: if set, wrap the whole body in a HW For_i loop that
    repeats it loop_reps times (timing harness only — result unchanged
    since the same X is re-read).
    mode: "full" (real kernel), "dma" (loads+stores only, no compute),
    "compute" (matmuls+copies on resident tiles, no X/Y DMA)."""
    nc = bacc.Bacc("TRN2", target_bir_lowering=False, debug=False)

    X = nc.dram_tensor("X", [ROWS_PER_CORE, N], F16, kind="ExternalInput")
    W1 = nc.dram_tensor("W1", [128, 128], F16, kind="ExternalInput")
    HJ2 = nc.dram_tensor("HJ2", [256, 256], F16, kind="ExternalInput")
    Y = nc.dram_tensor("Y", [ROWS_PER_CORE, N], F16, kind="ExternalOutput")

    # row r = 64*g + 8*a + b ; column c = 256*ip + jc
    # SBUF group tile: partition p = 16*b + ip, free f = 256*a + jc
    X_re = X[:].rearrange(
        "(g a b) (i j) -> g b i a j", a=A_PER_GROUP, b=8, i=16, j=256
    )
    Y_re = Y[:].rearrange(
        "(g a b) (i j) -> g b i a j", a=A_PER_GROUP, b=8, i=16, j=256
    )
    FREE = A_PER_GROUP * 256  # free size of a group tile

    with tile.TileContext(nc) as tc:
        with (
            tc.tile_pool(name="consts", bufs=1) as cpool,
            tc.tile_pool(name="xin", bufs=6) as xpool,
            tc.tile_pool(name="yout", bufs=6) as ypool,
            tc.tile_pool(name="mid", bufs=8) as spool,
            tc.tile_pool(name="psA", bufs=4, space="PSUM") as psA,
            tc.tile_pool(name="psB", bufs=4, space="PSUM") as psB,
        ):
            w1 = cpool.tile([128, 128], F16)
            nc.sync.dma_start(out=w1[:], in_=W1[:])
            # H256 split into its two 128-row slabs (contraction halves)
            # two distinct assignments: pool slots are tagged by source
            # variable name, and a bufs=1 pool deadlocks if two live tiles
            # share a tag (constants are never released)
            hj0 = cpool.tile([128, 256], F16)
            nc.sync.dma_start(out=hj0[:], in_=HJ2[0:128, :])
            hj1 = cpool.tile([128, 256], F16)
            nc.sync.dma_start(out=hj1[:], in_=HJ2[128:256, :])
            hj = [hj0, hj1]
            xconst = None
            if mode == "compute":
                xconst = cpool.tile([128, FREE], F16)
                nc.sync.dma_start(
                    out=xconst[:].rearrange("p (a j) -> p a j",
                                            a=A_PER_GROUP, j=256),
                    in_=X_re[0],
                )

            def flush_b(state):
                """Emit the B-stage (4 accumulating MM-B + ACT copy +
                maybe store) for a previously A-staged 8-row slice."""
                if state is None:
                    return
                sa, yw_3d_, yw_, u_, g_ = state
                pb = psB.tile([128, 512], F32)
                for s in range(2):
                    for ch in range(2):
                        for h in range(2):
                            nc.tensor.matmul(
                                pb[:, (2 * s + ch) * 128:(2 * s + ch + 1) * 128],
                                lhsT=sa[:, (2 * s + h) * 128:(2 * s + h + 1) * 128],
                                rhs=hj[h][:, ch * 128:(ch + 1) * 128],
                                start=(h == 0),
                                stop=(h == 1),
                            )
                nc.scalar.copy(
                    out=yw_[:, u_ * 512:(u_ + 1) * 512], in_=pb[:]
                )
                if u_ == A_PER_GROUP // 2 - 1 and mode != "compute":
                    # stores ride the ACT HWDGE ring; loads own the SP ring
                    nc.scalar.dma_start(out=Y_re[g_], in_=yw_3d_)

            def emit_body():
              # 1-stage software pipeline: each slice's MM-B block is
              # emitted after the NEXT slice's MM-A block, so the PE FIFO
              # never stalls on the DVE PSUM->SBUF copy in between.
              pending = []
              for g in range(GROUPS):
                if mode == "compute":
                    xw = xconst
                else:
                    xw = xpool.tile([128, FREE], F16)
                xw_3d = xw[:].rearrange("p (a j) -> p a j", a=A_PER_GROUP, j=256)
                if mode != "compute":
                    nc.sync.dma_start(out=xw_3d, in_=X_re[g])
                yw = ypool.tile([128, FREE], F16)
                yw_3d = yw[:].rearrange("p (a j) -> p a j", a=A_PER_GROUP, j=256)
                if mode == "dma":
                    nc.scalar.dma_start(out=Y_re[g], in_=xw_3d)
                    continue
                for u in range(A_PER_GROUP // 2):
                    # stage A for a PAIR of 8-row slices (full psum bank,
                    # one 512-wide DVE evacuation per 16 rows)
                    pa = psA.tile([128, 512], F32)
                    for s in range(2):
                        a = 2 * u + s
                        for h in range(2):
                            nc.tensor.matmul(
                                pa[:, (2 * s + h) * 128:(2 * s + h + 1) * 128],
                                lhsT=xw[:, a * 256 + h * 128:a * 256 + (h + 1) * 128],
                                rhs=w1[:],
                                start=True,
                                stop=True,
                            )
                    if len(pending) >= 2:
                        flush_b(pending.pop(0))
                    sa = spool.tile([128, 512], F16)
                    nc.vector.tensor_copy(out=sa[:], in_=pa[:])
                    pending.append((sa, yw_3d, yw, u, g))
              for st in pending:
                  flush_b(st)

            if loop_reps is None:
                emit_body()
            else:
                with tc.For_i(0, loop_reps, 1):
                    emit_body()

    nc.compile()
    return nc


_NC = None


def _get_nc():
    global _NC
    if _NC is None:
        _NC = _build_bass()
    return _NC


def make_in_maps(X: np.ndarray) -> list[dict]:
    """Shard X row-wise into 8 fp16 per-core input maps."""
    X = np.asarray(X, dtype=np.float32)
    assert X.shape == (ROWS, N), X.shape
    X16 = X.astype(np.float16)
    W1, HJ2 = _constants()
    return [
        {
            "X": X16[c * ROWS_PER_CORE:(c + 1) * ROWS_PER_CORE],
            "W1": W1,
            "HJ2": HJ2,
        }
        for c in range(N_CORES)
    ]


def run(X: np.ndarray, trace: bool = False):
    """Run the SPMD kernel on 8 cores; returns (Y, BassKernelResults)."""
    in_maps = make_in_maps(X)
    nc = _get_nc()
    res = run_bass_kernel_spmd(
        nc, in_maps, list(range(N_CORES)), trace=trace
    )
    Y = np.concatenate(
        [res.results[c]["Y"] for c in range(N_CORES)], axis=0
    ).astype(np.float32)
    return Y, res


def kernel(X, H=None, **_unused) -> np.ndarray:
    """Full-input entry point: X (8192, 4096) f32, H ignored (H is the
    deterministic normalized Hadamard matrix, synthesized on device)."""
    Y, _ = run(X, trace=False)
    return Y


# revision 32
# speedup vs baseline: 1.6050x; 1.6050x over previous
"""Trainium2 Bass kernel for nn_HadamardTransform: Y = X @ H4096_normalized.

Algorithm: H4096 (Sylvester, normalized) factors exactly as the Kronecker
product H16n (x) H256n.  Each row x of X, reshaped row-major to R[16, 256],
transforms as  Y_mat = G16 @ R @ H256u  with G16 = 2^-6 * H16u (all of the
2^-6 normalization folded into the 16-side so H256u stays exactly +-1).

On-chip scheme per pair of 8-row slices (partition p = 16*b + ip, b in
[8] rows, ip in [16] column superblocks of 256; free = (a, jc in [256]);
pairing keeps every PSUM tile a full 512-wide bank and every PSUM
evacuation one 512-col engine op):
  MM-A (x4, one per slice s and 128-half jh of jc):
      psumA[2s+jh] = T_s_jh.T @ W1      (W1 = I8 (x) G16, block-diagonal)
      -> psumA[jc, (b,i')] : the 16-transform, emerging jc-on-partitions
  MM-B (x8, accumulating pairs over jh):
      psumB[:, 2s+ch] = sum_h sa[2s+h].T @ H256[h, ch]
      -> psumB[(b,i'), jc'] : the 256-transform, natural output layout
No transposes are needed anywhere; the fixed matrices are the moving
operands, the per-slice data is the stationary operand.

Why 16x256 rather than 32x128: the DMA descriptor economy.  Both layouts
move the same bytes, but contiguous runs here are 512 B (256 fp16) vs
256 B, halving descriptor count; measured on HW the 8-core DMA floor
drops from ~67 us to ~60.5 us (payload-bound at ~278 GB/s/core), at the
cost of a third PE pass (the 256-contraction needs two accumulating
matmuls).  PE and DMA both land near 60 us and overlap.

Everything on device is fp16: X is cast host-side (rel err ~5e-4 vs the
2e-2 gate), halving DMA traffic (the roofline here); fp16 moving
operands stream at 1 cycle/column through the PE vs 4 for fp32.  PSUM
accumulation stays fp32; the PSUM->SBUF evacuations cast to fp16 (DVE
for stage A, ACT for stage B).  Loads ride the SP HWDGE ring, stores the
ACT HWDGE ring.

Sharding: X's 8192 rows split into 8 contiguous shards of 1024 rows, one
per NeuronCore (pure data parallelism, no collectives).
"""

import sys

import numpy as np

try:
    import concourse.bass as bass
except ImportError:  # repo not on sys.path in a fresh grading dir
    sys.path.insert(0, "/opt/trn_rl_repo")
    import concourse.bass as bass

import concourse.mybir as mybir
import concourse.tile as tile
from concourse import bacc
from concourse.bass_utils import run_bass_kernel_spmd

N_CORES = 8
ROWS = 8192
N = 4096
ROWS_PER_CORE = ROWS // N_CORES  # 1024
ROWS_PER_GROUP = 64               # rows moved per DMA (512 KiB fp16)
A_PER_GROUP = ROWS_PER_GROUP // 8  # 8-row slices per group
GROUPS = ROWS_PER_CORE // ROWS_PER_GROUP  # 16
F16 = mybir.dt.float16
F32 = mybir.dt.float32


def _hadamard_u(n: int) -> np.ndarray:
    """Unnormalized Sylvester Hadamard matrix (+-1 entries)."""
    H = np.array([[1.0]], dtype=np.float64)
    while H.shape[0] < n:
        H = np.block([[H, H], [H, -H]])
    return H


def _constants() -> tuple[np.ndarray, np.ndarray]:
    G16 = (2.0 ** -6) * _hadamard_u(16)         # fold full 2^-6 norm here
    W1 = np.kron(np.eye(8), G16).astype(np.float16)  # [128,128] block-diag
    HJ2 = _hadamard_u(256).astype(np.float16)        # [256,256] exact +-1
    return W1, HJ2


def _build_bass(loop_reps: int | None = None, mode: str = "full"):
    """loop_reps: if set, wrap the whole body in a HW For_i loop that
    repeats it loop_reps times (timing harness only — result unchanged
    since the same X is re-read).
    mode: "full" (real kernel), "dma" (loads+stores only, no compute),
    "compute" (matmuls+copies on resident tiles, no X/Y DMA)."""
    nc = bacc.Bacc("TRN2", target_bir_lowering=False, debug=False)

    X = nc.dram_tensor("X", [ROWS_PER_CORE, N], F16, kind="ExternalInput")
    W1 = nc.dram_tensor("W1", [128, 128], F16, kind="ExternalInput")
    HJ2 = nc.dram_tensor("HJ2", [256, 256], F16, kind="ExternalInput")
    Y = nc.dram_tensor("Y", [ROWS_PER_CORE, N], F16, kind="ExternalOutput")

    # row r = 64*g + 8*a + b ; column c = 256*ip + jc
    # SBUF group tile: partition p = 16*b + ip, free f = 256*a + jc
    X_re = X[:].rearrange(
        "(g a b) (i j) -> g b i a j", a=A_PER_GROUP, b=8, i=16, j=256
    )
    Y_re = Y[:].rearrange(
        "(g a b) (i j) -> g b i a j", a=A_PER_GROUP, b=8, i=16, j=256
    )
    FREE = A_PER_GROUP * 256  # free size of a group tile

    with tile.TileContext(nc) as tc:
        with (
            tc.tile_pool(name="consts", bufs=1) as cpool,
            tc.tile_pool(name="xin", bufs=5) as xpool,
            tc.tile_pool(name="yout", bufs=4) as ypool,
            tc.tile_pool(name="mid", bufs=8) as spool,
            tc.tile_pool(name="psA", bufs=4, space="PSUM") as psA,
            tc.tile_pool(name="psB", bufs=4, space="PSUM") as psB,
        ):
            w1 = cpool.tile([128, 128], F16)
            nc.sync.dma_start(out=w1[:], in_=W1[:])
            # H256 split into its two 128-row slabs (contraction halves)
            # two distinct assignments: pool slots are tagged by source
            # variable name, and a bufs=1 pool deadlocks if two live tiles
            # share a tag (constants are never released)
            hj0 = cpool.tile([128, 256], F16)
            nc.sync.dma_start(out=hj0[:], in_=HJ2[0:128, :])
            hj1 = cpool.tile([128, 256], F16)
            nc.sync.dma_start(out=hj1[:], in_=HJ2[128:256, :])
            hj = [hj0, hj1]
            xconst = None
            if mode == "compute":
                xconst = cpool.tile([128, FREE], F16)
                nc.sync.dma_start(
                    out=xconst[:].rearrange("p (a j) -> p a j",
                                            a=A_PER_GROUP, j=256),
                    in_=X_re[0],
                )

            def flush_b(state):
                """Emit the B-stage (4 accumulating MM-B + ACT copy +
                maybe store) for a previously A-staged 8-row slice."""
                if state is None:
                    return
                sa, yw_3d_, yw_, u_, g_ = state
                pb = psB.tile([128, 512], F32)
                for s in range(2):
                    for h in range(2):
                        # one 256-col moving stream per (s, h): half the
                        # LDWEIGHTS/instruction count of 128-col chunks
                        nc.tensor.matmul(
                            pb[:, s * 256:(s + 1) * 256],
                            lhsT=sa[:, (2 * s + h) * 128:(2 * s + h + 1) * 128],
                            rhs=hj[h][:],
                            start=(h == 0),
                            stop=(h == 1),
                        )
                nc.scalar.copy(
                    out=yw_[:, u_ * 512:(u_ + 1) * 512], in_=pb[:]
                )
                if u_ == A_PER_GROUP // 2 - 1 and mode != "compute":
                    # stores ride the ACT HWDGE ring; loads own the SP ring
                    nc.scalar.dma_start(out=Y_re[g_], in_=yw_3d_)

            def emit_body():
              # 1-stage software pipeline: each slice's MM-B block is
              # emitted after the NEXT slice's MM-A block, so the PE FIFO
              # never stalls on the DVE PSUM->SBUF copy in between.
              pending = []
              for g in range(GROUPS):
                if mode == "compute":
                    xw = xconst
                else:
                    xw = xpool.tile([128, FREE], F16)
                xw_3d = xw[:].rearrange("p (a j) -> p a j", a=A_PER_GROUP, j=256)
                if mode != "compute":
                    nc.sync.dma_start(out=xw_3d, in_=X_re[g])
                yw = ypool.tile([128, FREE], F16)
                yw_3d = yw[:].rearrange("p (a j) -> p a j", a=A_PER_GROUP, j=256)
                if mode == "dma":
                    nc.scalar.dma_start(out=Y_re[g], in_=xw_3d)
                    continue
                for u in range(A_PER_GROUP // 2):
                    # stage A for a PAIR of 8-row slices (full psum bank,
                    # one 512-wide DVE evacuation per 16 rows)
                    pa = psA.tile([128, 512], F32)
                    for s in range(2):
                        a = 2 * u + s
                        for h in range(2):
                            nc.tensor.matmul(
                                pa[:, (2 * s + h) * 128:(2 * s + h + 1) * 128],
                                lhsT=xw[:, a * 256 + h * 128:a * 256 + (h + 1) * 128],
                                rhs=w1[:],
                                start=True,
                                stop=True,
                            )
                    if len(pending) >= 2:
                        flush_b(pending.pop(0))
                    sa = spool.tile([128, 512], F16)
                    nc.vector.tensor_copy(out=sa[:], in_=pa[:])
                    pending.append((sa, yw_3d, yw, u, g))
              for st in pending:
                  flush_b(st)

            if loop_reps is None:
                emit_body()
            else:
                with tc.For_i(0, loop_reps, 1):
                    emit_body()

    nc.compile()
    return nc


_NC = None


def _get_nc():
    global _NC
    if _NC is None:
        _NC = _build_bass()
    return _NC


def make_in_maps(X: np.ndarray) -> list[dict]:
    """Shard X row-wise into 8 fp16 per-core input maps."""
    X = np.asarray(X, dtype=np.float32)
    assert X.shape == (ROWS, N), X.shape
    X16 = X.astype(np.float16)
    W1, HJ2 = _constants()
    return [
        {
            "X": X16[c * ROWS_PER_CORE:(c + 1) * ROWS_PER_CORE],
            "W1": W1,
            "HJ2": HJ2,
        }
        for c in range(N_CORES)
    ]


def run(X: np.ndarray, trace: bool = False):
    """Run the SPMD kernel on 8 cores; returns (Y, BassKernelResults)."""
    in_maps = make_in_maps(X)
    nc = _get_nc()
    res = run_bass_kernel_spmd(
        nc, in_maps, list(range(N_CORES)), trace=trace
    )
    Y = np.concatenate(
        [res.results[c]["Y"] for c in range(N_CORES)], axis=0
    ).astype(np.float32)
    return Y, res


def kernel(X, H=None, **_unused) -> np.ndarray:
    """Full-input entry point: X (8192, 4096) f32, H ignored (H is the
    deterministic normalized Hadamard matrix, synthesized on device)."""
    Y, _ = run(X, trace=False)
    return Y
